# revision 1
# baseline (speedup 1.0000x reference)
"""BitLinear (ternary-quantized linear) Trainium2 kernel.

Computes: scale = clip(mean(|w|, axis=1), 1e-5);  w_q = clip(round(w/scale), -1, 1)
          out = x @ (w_q * scale).T
for x [4, 2048, 2048] f32, w [8192, 2048] f32, out [4, 2048, 8192] f32.

Strategy (8 NeuronCores, tensor-parallel over weight rows / out_features):
  - Each core gets a 1024-row shard of w and a full copy of x (fed pre-transposed
    [d_in, tokens] so the contraction dim lands on SBUF partitions; transposition
    is a host-side layout choice only - all arithmetic happens on device).
  - On device: quantize w rows exactly in fp32. The per-row scale uses a
    blocked-512 two-stage reduction, which reproduces bit-for-bit the
    neuronxcc-lowered jnp.mean of the reference, and
    w_q = (w > scale/2) - (w < -scale/2), which equals clip(round(w/scale),-1,1)
    exactly (round is round-half-even).
  - x is cast to bf16 on the scalar engine; the matmul runs in bf16 (ternary
    w_q is exact in bf16) with fp32 PSUM accumulation; the per-o scale is
    applied in the PSUM->SBUF epilogue on the vector engine.
  - w_q is transposed to [d_in, o] layout on the tensor engine during prologue
    gaps; the weight prologue is split in two o-halves with the first token
    slab's first-half matmuls emitted in between, so the PE starts real work
    while the second half is still quantizing.
  - Output stays o-sharded per core; host concatenates.
"""

import os

import numpy as np

B, S, D_IN, D_OUT = 4, 2048, 2048, 8192
T = B * S  # 8192 tokens
N_CORES = 8
O_SHARD = D_OUT // N_CORES  # 1024
EPS = 1e-05

P = 128
KC = D_IN // P  # 16 contraction chunks
N_OT = O_SHARD // P  # 8 o-tiles per core
T_SLAB = 512  # tokens per x slab kept in SBUF
N_SLABS = T // T_SLAB  # 16
TSUB = T_SLAB // P  # 4 psum blocks per slab
OHALF = O_SHARD // 2  # 512, matmul free dim / psum bank

# knobs (env-tunable for experiments)
USE_DMA_CAST = os.environ.get("BL_DMA_CAST", "0") == "1"
EARLY_SLABS = int(os.environ.get("BL_EARLY_SLABS", "0"))

_CACHE = {}


def _build_program():
    import concourse.bass as bass
    import concourse.tile as tile
    from concourse import bacc, mybir
    from concourse.masks import make_identity

    f32 = mybir.dt.float32
    bf16 = mybir.dt.bfloat16

    nc = bacc.Bacc(
        "TRN2",
        target_bir_lowering=False,
        debug=False,
        num_devices=N_CORES,
    )

    xT = nc.dram_tensor("xT", [D_IN, T], f32, kind="ExternalInput")
    w = nc.dram_tensor("w", [O_SHARD, D_IN], f32, kind="ExternalInput")
    out = nc.dram_tensor("out", [T, O_SHARD], f32, kind="ExternalOutput")

    xT3 = xT.ap().rearrange("(c p) t -> p c t", p=P)  # [128, 16, 8192]

    with tile.TileContext(nc) as tc:
        const_pool = tc.alloc_tile_pool(name="const", bufs=1)
        wqt_pool = tc.alloc_tile_pool(name="wq_T", bufs=1)
        sb_pool = tc.alloc_tile_pool(name="scaleB", bufs=1)
        w_pool = tc.alloc_tile_pool(name="wstage", bufs=2)
        wq_pool = tc.alloc_tile_pool(name="wq", bufs=2)
        st_pool = tc.alloc_tile_pool(name="stats", bufs=N_OT)
        psum_pro = tc.alloc_tile_pool(name="psum_pro", bufs=1, space="PSUM")
        xb_pool = tc.alloc_tile_pool(name="xb", bufs=3)
        xf_pool = tc.alloc_tile_pool(name="xf", bufs=3)
        out_pool = tc.alloc_tile_pool(name="osb", bufs=4)
        psum_mm = tc.alloc_tile_pool(name="psum_mm", bufs=2, space="PSUM")
        dram_pool = tc.alloc_tile_pool(name="dram", bufs=1, space="DRAM")
        ctx_pools = [const_pool, wqt_pool, sb_pool, w_pool, wq_pool, st_pool,
                     psum_pro, xb_pool, xf_pool, out_pool, psum_mm, dram_pool]

        ident_bf = const_pool.tile([P, P], bf16)
        make_identity(nc, ident_bf)
        ident_f32 = const_pool.tile([P, P], f32)
        make_identity(nc, ident_f32)
        ones_f32 = const_pool.tile([P, P], f32)
        nc.vector.memset(ones_f32[:], 1.0)

        # resident: transposed ternary weights (one tile per o-half) and
        # the per-o scale broadcast across all 128 partitions
        wqTh = [wqt_pool.tile([P, KC, OHALF], bf16, tag=f"wqT{h}",
                              name=f"wqT{h}")
                for h in range(2)]
        scaleB = sb_pool.tile([P, O_SHARD], f32)
        wq_dram = dram_pool.tile([O_SHARD, D_IN], bf16)

        def prologue_otile(ot, via_dma=False):
            """Quantize o-tile `ot` of w and transpose it into wqTh."""
            wf = w_pool.tile([P, D_IN], f32, name="wf")
            nc.sync.dma_start(wf[:], w[bass.ts(ot, P), :])

            # blocked-512 two-stage reduce: bit-exact match with the
            # neuronxcc-lowered jnp.mean the reference runs through
            ssum4 = st_pool.tile([P, 4], f32, tag="ssum4", name="ssum4")
            nc.vector.tensor_reduce(
                out=ssum4[:],
                in_=wf[:].rearrange("p (b k) -> p b k", k=512),
                op=mybir.AluOpType.add,
                axis=mybir.AxisListType.X,
                apply_absolute_value=True,
            )
            ssum = st_pool.tile([P, 1], f32, tag="ssum", name="ssum")
            nc.vector.tensor_reduce(
                out=ssum[:], in_=ssum4[:],
                op=mybir.AluOpType.add, axis=mybir.AxisListType.X,
            )
            scale = st_pool.tile([P, 1], f32, tag="scale", name="scale")
            nc.vector.tensor_scalar(
                scale[:], ssum[:], 1.0 / D_IN, EPS,
                mybir.AluOpType.mult, mybir.AluOpType.max,
            )
            thr = st_pool.tile([P, 1], f32, tag="thr", name="thr")
            nc.vector.tensor_scalar_mul(thr[:], scale[:], 0.5)
            nthr = st_pool.tile([P, 1], f32, tag="nthr", name="nthr")
            nc.vector.tensor_scalar_mul(nthr[:], thr[:], -1.0)

            # w_q = (w > thr) - (w < -thr)  in {-1, 0, 1}, exact in bf16
            neg = wq_pool.tile([P, D_IN], f32, tag="neg", name="neg")
            nc.vector.tensor_scalar(
                neg[:], wf[:], nthr[:], None, mybir.AluOpType.is_lt,
            )
            wq = wq_pool.tile([P, D_IN], bf16, tag="wq", name="wq")
            nc.vector.scalar_tensor_tensor(
                out=wq[:], in0=wf[:], scalar=thr[:], in1=neg[:],
                op0=mybir.AluOpType.is_gt, op1=mybir.AluOpType.subtract,
            )

            # transpose wq [o,i] -> wqT [i,o]
            h, col = divmod(ot * P, OHALF)
            if via_dma:
                # park wq in DRAM; the xbar transpose reads it back later
                nc.sync.dma_start(wq_dram[bass.ts(ot, P), :], wq[:])
            else:
                for kc in range(KC):
                    pt = psum_pro.tile([P, P], bf16, tag="tp", name="pt",
                                       bufs=3)
                    nc.tensor.transpose(pt[:], wq[:, bass.ts(kc, P)],
                                        ident_bf[:])
                    nc.scalar.copy(out=wqTh[h][:, kc, bass.ds(col, P)],
                                   in_=pt[:])

            # scaleB[:, ot*128:+128] = scale broadcast over partitions:
            # ones.T @ diag(scale)
            ds_t = wq_pool.tile([P, P], f32, tag="diag", name="ds_t")
            nc.vector.tensor_scalar(
                ds_t[:], ident_f32[:], scale[:], None, mybir.AluOpType.mult,
            )
            bp = psum_pro.tile([P, P], f32, tag="bp", name="bp", bufs=1)
            nc.tensor.matmul(bp[:], ones_f32[:], ds_t[:], start=True, stop=True)
            nc.scalar.copy(out=scaleB[:, bass.ts(ot, P)], in_=bp[:])

        xb_tiles = {}

        def load_slab(s):
            tsl = bass.ts(s, T_SLAB)
            xb = xb_pool.tile([P, KC, T_SLAB], bf16, name="xb")
            if USE_DMA_CAST:
                nc.gpsimd.dma_start(xb[:], xT3[:, :, tsl])
            else:
                for q in range(4):
                    xf = xf_pool.tile([P, KC // 4, T_SLAB], f32, name="xf")
                    nc.sync.dma_start(xf[:], xT3[:, bass.ts(q, KC // 4), tsl])
                    nc.scalar.copy(out=xb[:, bass.ts(q, KC // 4), :], in_=xf[:])
            xb_tiles[s] = xb

        osb_tiles = {}

        def mm_half(s, tsub, h):
            """Matmuls + scale epilogue for one 128-token block, one o-half."""
            xb = xb_tiles[s]
            ps = psum_mm.tile([P, OHALF], mybir.dt.float32, tag=f"ps{h}",
                              name="ps")
            for kc in range(KC):
                nc.tensor.matmul(
                    ps[:],
                    xb[:, kc, bass.ts(tsub, P)],
                    wqTh[h][:, kc, :],
                    start=(kc == 0),
                    stop=(kc == KC - 1),
                )
            osb = out_pool.tile([P, OHALF], f32, tag=f"osb{h}", name="osb")
            nc.vector.tensor_tensor(
                osb[:], ps[:], scaleB[:, bass.ts(h, OHALF)],
                mybir.AluOpType.mult,
            )
            row0 = (s * TSUB + tsub) * P
            nc.sync.dma_start(
                out[bass.ds(row0, P), bass.ts(h, OHALF)], osb[:]
            )

        def store_block(s, tsub):
            pass

        # ---------------- emission schedule -----------------------------
        # Half 0 of w transposes on the PE (hidden under the DVE quant of
        # half 1); half 1 goes through DRAM + the DMA xbar transpose, which
        # overlaps the first slabs' half-0 matmuls. All PE prologue ops
        # stay strictly before the first matmul (interleaving PE transposes
        # between matmul groups faults the hardware).
        nE = max(0, min(EARLY_SLABS, N_SLABS))
        if nE == 0:
            for ot in range(N_OT):
                prologue_otile(ot)
            for s in range(N_SLABS):
                load_slab(s)
                for tsub in range(TSUB):
                    mm_half(s, tsub, 0)
                    mm_half(s, tsub, 1)
                    store_block(s, tsub)
        else:
            # interleave: half-0 prologue, early half-0 matmuls, half-1
            # prologue, rest. PE drains flush the LDWEIGHTS reorder window
            # at every transpose-mode <-> matmul-mode transition (the
            # window otherwise pulls a transpose-mode LDWEIGHTS ahead of
            # in-flight matmuls, which faults the exec unit).
            for s in range(nE):
                load_slab(s)
            for ot in range(N_OT // 2):
                prologue_otile(ot)
            nc.tensor.drain()
            for s in range(nE):
                for tsub in range(TSUB):
                    mm_half(s, tsub, 0)
            nc.tensor.drain()
            for ot in range(N_OT // 2, N_OT):
                prologue_otile(ot)
            nc.tensor.drain()
            for s in range(nE):
                for tsub in range(TSUB):
                    mm_half(s, tsub, 1)
                    store_block(s, tsub)
            for s in range(nE, N_SLABS):
                load_slab(s)
                for tsub in range(TSUB):
                    mm_half(s, tsub, 0)
                    mm_half(s, tsub, 1)
                    store_block(s, tsub)

        for p in reversed(ctx_pools):
            p.release()

    nc.compile()
    return nc


def _get_program():
    if "nc" not in _CACHE:
        _CACHE["nc"] = _build_program()
    return _CACHE["nc"]


def _ensure_ntff_hook():
    """Provide antenv.axon_hooks if the image lacks it (profiling only)."""
    import sys
    import types

    try:
        from antenv.axon_hooks import get_axon_ntff_profile_hook  # noqa: F401
        return
    except ImportError:
        pass
    try:
        import antenv
        from trn_agent_boot.trn_boot import _ntff_profile_via_ctypes

        mod = types.ModuleType("antenv.axon_hooks")
        state = {"hook": _ntff_profile_via_ctypes("/opt/axon/libaxon_pjrt.so")}
        mod.get_axon_ntff_profile_hook = lambda: state["hook"]
        mod.set_axon_ntff_profile_hook = lambda h: state.__setitem__("hook", h)
        sys.modules["antenv.axon_hooks"] = mod
        antenv.axon_hooks = mod
    except Exception:
        pass


def kernel(x: np.ndarray, weight: np.ndarray) -> np.ndarray:
    from concourse.bass_utils import run_bass_kernel_spmd

    assert x.shape == (B, S, D_IN) and weight.shape == (D_OUT, D_IN)
    nc = _get_program()

    xT = np.ascontiguousarray(x.reshape(T, D_IN).T)
    in_maps = [
        {"xT": xT, "w": weight[c * O_SHARD : (c + 1) * O_SHARD]}
        for c in range(N_CORES)
    ]

    trace = os.environ.get("BL_TRACE", "0") == "1"
    if trace:
        _ensure_ntff_hook()
    res = run_bass_kernel_spmd(nc, in_maps, list(range(N_CORES)), trace=trace)
    _CACHE["last_results"] = res

    parts = [res.results[c]["out"] for c in range(N_CORES)]
    full = np.concatenate(parts, axis=1)  # [T, D_OUT]
    return np.ascontiguousarray(full.reshape(B, S, D_OUT)).astype(np.float32, copy=False)



# revision 2
# speedup vs baseline: 1.2580x; 1.2580x over previous
"""BitLinear (ternary-quantized linear) Trainium2 kernel, v2: fp8 DoubleRow.

Computes: scale = clip(mean(|w|, axis=1), 1e-5);  w_q = clip(round(w/scale), -1, 1)
          out = x @ (w_q * scale).T
for x [4, 2048, 2048] f32, w [8192, 2048] f32, out [4, 2048, 8192] f32.

Strategy (8 NeuronCores, tensor-parallel over weight rows / out_features):
  - Each core gets a 1024-row shard of w and a full copy of x. Host feeds x
    transposed [d_in, tokens] and the w shard in BOTH layouts ([o, k] for the
    bit-exact per-row scale reduction on the DVE, [k, o] for quantization in
    matmul orientation) - layout choices only, all arithmetic is on device.
  - Per-row scale uses the blocked-512 two-stage reduction (bit-exact match
    with the reference's jnp.mean lowering); w_q = (w > s/2) - (w < -s/2)
    equals clip(round(w/scale), -1, 1) exactly. w_q is ternary {-1,0,1},
    exact in fp8e4 and bf16.
  - The contraction (2048 = 16 chunks of 128) is split: the first BL_FP8_KC
    (default 14) chunks run as fp8e4 DoubleRow matmuls (2 chunks/matmul, 2x
    PE rate), the rest as bf16 matmuls. x is cast f32->fp8e4/bf16 on the
    scalar engine per 512-token slab. Measured-on-host quantization error of
    this split is ~1.8e-2 scale-relative (gate 2e-2); the bf16 tail chunks
    keep margin.
  - Per-o scale applied in the PSUM->SBUF epilogue (vector engine), output
    stored bf16 (host upcasts); output stays o-sharded, host concatenates.
  - Weight prologue is split by o-half: half 0 quantizes first, then the
    first BL_EARLY slabs' half-0 matmuls run on the PE while half 1's scale/
    quantization interleaves on the DVE between those groups.
"""

import os

import numpy as np

B, S, D_IN, D_OUT = 4, 2048, 2048, 8192
T = B * S  # 8192 tokens
N_CORES = 8
O_SHARD = D_OUT // N_CORES  # 1024
EPS = 1e-05

P = 128
KC = D_IN // P  # 16 contraction chunks
T_SLAB = 512  # tokens per x slab kept in SBUF
N_SLABS = T // T_SLAB  # 16
TSUB = T_SLAB // P  # 4 psum blocks per slab
OHALF = O_SHARD // 2  # 512, matmul free dim / psum bank

# knobs (env-tunable for experiments)
NF8 = int(os.environ.get("BL_FP8_KC", "14"))  # k-chunks through fp8 DoubleRow
assert 0 <= NF8 <= KC and NF8 % 2 == 0
NB16 = KC - NF8
NPAIR = NF8 // 2
EARLY = int(os.environ.get("BL_EARLY", "3"))  # slabs whose h0 runs pre-h1-quant
EARLY = max(1, min(EARLY, N_SLABS))

_CACHE = {}


def _build_program():
    import concourse.bass as bass
    import concourse.tile as tile
    from concourse import bacc, mybir
    from concourse.masks import make_identity

    f32 = mybir.dt.float32
    bf16 = mybir.dt.bfloat16
    fp8 = mybir.dt.float8e4
    DR = mybir.MatmulPerfMode.DoubleRow

    nc = bacc.Bacc(
        "TRN2",
        target_bir_lowering=False,
        debug=False,
        num_devices=N_CORES,
    )

    xT = nc.dram_tensor("xT", [D_IN, T], f32, kind="ExternalInput")
    w = nc.dram_tensor("w", [O_SHARD, D_IN], f32, kind="ExternalInput")
    wT = nc.dram_tensor("wT", [D_IN, O_SHARD], f32, kind="ExternalInput")
    out = nc.dram_tensor("out", [T, O_SHARD], bf16, kind="ExternalOutput")

    xT3 = xT.ap().rearrange("(c p) t -> p c t", p=P)  # [128, 16, 8192]
    wT3 = wT.ap().rearrange("(c p) o -> p c o", p=P)  # [128, 16, 1024]

    with tile.TileContext(nc) as tc:
        const_pool = tc.alloc_tile_pool(name="const", bufs=1)
        wq8_pool = tc.alloc_tile_pool(name="wq8", bufs=1)
        wq16_pool = tc.alloc_tile_pool(name="wq16", bufs=1)
        sb_pool = tc.alloc_tile_pool(name="scaleB", bufs=1)
        w_pool = tc.alloc_tile_pool(name="wstage", bufs=3)
        wt_pool = tc.alloc_tile_pool(name="wtstage", bufs=3)
        gl_pool = tc.alloc_tile_pool(name="gl", bufs=2)
        st_pool = tc.alloc_tile_pool(name="stats", bufs=4)
        psum_pro = tc.alloc_tile_pool(name="psum_pro", bufs=1, space="PSUM")
        xb_pool = tc.alloc_tile_pool(name="xb", bufs=EARLY + 1)
        xf_pool = tc.alloc_tile_pool(name="xf", bufs=3)
        out_pool = tc.alloc_tile_pool(name="osb", bufs=3)
        outh_pool = tc.alloc_tile_pool(name="osbh", bufs=4)
        psum_mm = tc.alloc_tile_pool(name="psum_mm", bufs=2, space="PSUM")
        ctx_pools = [const_pool, wq8_pool, wq16_pool, sb_pool, w_pool, wt_pool,
                     gl_pool, st_pool, psum_pro, xb_pool, xf_pool, out_pool,
                     outh_pool, psum_mm]

        ident_f32 = const_pool.tile([P, P], f32)
        make_identity(nc, ident_f32)
        ones_f32 = const_pool.tile([P, P], f32)
        nc.vector.memset(ones_f32[:], 1.0)

        # resident: ternary weights in [k, o] matmul orientation (fp8 chunks
        # + bf16 tail chunks) and the per-o scale broadcast across partitions
        wqT8 = wq8_pool.tile([P, NF8, O_SHARD], fp8, name="wqT8")
        wqT16 = (wq16_pool.tile([P, NB16, O_SHARD], bf16, name="wqT16")
                 if NB16 else None)
        scaleB = sb_pool.tile([P, O_SHARD], f32, tag="scaleB", name="scaleB")
        thrB = sb_pool.tile([P, O_SHARD], f32, tag="thrB", name="thrB")
        nthrB = sb_pool.tile([P, O_SHARD], f32, tag="nthrB", name="nthrB")

        def scale_otile(ot):
            """Per-row scale for o-tile `ot`, broadcast into scaleB."""
            wf = w_pool.tile([P, D_IN], f32, name="wf")
            nc.sync.dma_start(wf[:], w[bass.ts(ot, P), :])

            # blocked-512 two-stage reduce: bit-exact match with the
            # reference's lowered jnp.mean
            ssum4 = st_pool.tile([P, 4], f32, tag="ssum4", name="ssum4")
            nc.vector.tensor_reduce(
                out=ssum4[:],
                in_=wf[:].rearrange("p (b k) -> p b k", k=512),
                op=mybir.AluOpType.add,
                axis=mybir.AxisListType.X,
                apply_absolute_value=True,
            )
            ssum = st_pool.tile([P, 1], f32, tag="ssum", name="ssum")
            nc.vector.tensor_reduce(
                out=ssum[:], in_=ssum4[:],
                op=mybir.AluOpType.add, axis=mybir.AxisListType.X,
            )
            scale = st_pool.tile([P, 1], f32, tag="scale", name="scale")
            nc.vector.tensor_scalar(
                scale[:], ssum[:], 1.0 / D_IN, EPS,
                mybir.AluOpType.mult, mybir.AluOpType.max,
            )

            # scaleB[:, ot*128:+128] = scale broadcast over partitions:
            # ones.T @ diag(scale)
            ds_t = st_pool.tile([P, P], f32, tag="diag", name="ds_t")
            nc.vector.tensor_scalar(
                ds_t[:], ident_f32[:], scale[:], None, mybir.AluOpType.mult,
            )
            bp = psum_pro.tile([P, P], f32, tag="bp", name="bp", bufs=1)
            nc.tensor.matmul(bp[:], ones_f32[:], ds_t[:], start=True, stop=True)
            nc.scalar.copy(out=scaleB[:, bass.ts(ot, P)], in_=bp[:])

        def make_thr(h):
            hc = bass.ds(h * OHALF, OHALF)
            nc.vector.tensor_scalar_mul(thrB[:, hc], scaleB[:, hc], 0.5)
            nc.vector.tensor_scalar_mul(nthrB[:, hc], thrB[:, hc], -1.0)

        def quant_tile(h, kc):
            """Quantize wT chunk kc, o-columns of half h, into wqT8/wqT16."""
            hc = bass.ds(h * OHALF, OHALF)
            wtf = wt_pool.tile([P, OHALF], f32, name="wtf")
            nc.sync.dma_start(wtf[:], wT3[:, kc, hc])
            g = gl_pool.tile([P, OHALF], f32, tag="g", name="g")
            nc.vector.tensor_tensor(g[:], wtf[:], thrB[:, hc],
                                    mybir.AluOpType.is_gt)
            l = gl_pool.tile([P, OHALF], f32, tag="l", name="l")
            nc.vector.tensor_tensor(l[:], wtf[:], nthrB[:, hc],
                                    mybir.AluOpType.is_lt)
            dst = wqT8[:, kc, hc] if kc < NF8 else wqT16[:, kc - NF8, hc]
            nc.vector.tensor_tensor(dst, g[:], l[:], mybir.AluOpType.subtract)

        xb_tiles = {}

        def load_slab(s):
            tsl = bass.ts(s, T_SLAB)
            xb8 = xb_pool.tile([P, NF8, T_SLAB], fp8, tag="xb8", name="xb8")
            xb16 = (xb_pool.tile([P, NB16, T_SLAB], bf16, tag="xb16",
                                 name="xb16") if NB16 else None)
            for q in range(4):
                xf = xf_pool.tile([P, 4, T_SLAB], f32, name="xf")
                nc.sync.dma_start(xf[:], xT3[:, bass.ts(q, 4), tsl])
                base = q * 4
                n8 = max(0, min(4, NF8 - base))
                if n8:
                    nc.scalar.copy(out=xb8[:, bass.ds(base, n8), :],
                                   in_=xf[:, bass.ds(0, n8), :])
                if n8 < 4:
                    nc.scalar.copy(
                        out=xb16[:, bass.ds(base + n8 - NF8, 4 - n8), :],
                        in_=xf[:, bass.ds(n8, 4 - n8), :])
            xb_tiles[s] = (xb8, xb16)

        def mm_group(s, tsub, h, osb_full=None):
            """Matmuls + scale epilogue for one 128-token block, one o-half."""
            xb8, xb16 = xb_tiles[s]
            tok = bass.ts(tsub, P)
            hc = bass.ds(h * OHALF, OHALF)
            ps = psum_mm.tile([P, OHALF], f32, tag=f"ps{h}", name="ps")
            n_mm = NPAIR + NB16
            idx = 0
            for kp in range(NPAIR):
                nc.tensor.matmul(
                    ps[:],
                    xb8[:, bass.ds(2 * kp, 2), tok],
                    wqT8[:, bass.ds(2 * kp, 2), hc],
                    start=(idx == 0),
                    stop=(idx == n_mm - 1),
                    perf_mode=DR,
                )
                idx += 1
            for j in range(NB16):
                nc.tensor.matmul(
                    ps[:],
                    xb16[:, j, tok],
                    wqT16[:, j, hc],
                    start=(idx == 0),
                    stop=(idx == n_mm - 1),
                )
                idx += 1
            row0 = (s * TSUB + tsub) * P
            if osb_full is None:
                osb = outh_pool.tile([P, OHALF], bf16, name="osbh")
                nc.vector.tensor_tensor(osb[:], ps[:], scaleB[:, hc],
                                        mybir.AluOpType.mult)
                nc.sync.dma_start(out[bass.ds(row0, P), hc], osb[:])
            else:
                nc.vector.tensor_tensor(osb_full[:, hc], ps[:], scaleB[:, hc],
                                        mybir.AluOpType.mult)

        # ---------------- emission schedule -----------------------------
        # A: scale + thresholds + quantized weights for o-half 0
        for ot in range(4):
            scale_otile(ot)
        make_thr(0)
        for kc in range(KC):
            quant_tile(0, kc)

        # B: early slabs' half-0 groups, with half-1 prologue work (scale
        # o-tiles 4-7, thresholds, quant) interleaved so the DVE fills the
        # gaps between epilogues while the PE streams matmuls.
        for s in range(EARLY):
            load_slab(s)
        c_tasks = [lambda ot=ot: scale_otile(ot) for ot in range(4, 8)]
        c_tasks.append(lambda: make_thr(1))
        c_tasks += [lambda kc=kc: quant_tile(1, kc) for kc in range(KC)]
        groups = [(s, tsub) for s in range(EARLY) for tsub in range(TSUB)]
        per = (len(c_tasks) + len(groups) - 1) // len(groups)
        ci = 0
        for (s, tsub) in groups:
            for _ in range(per):
                if ci < len(c_tasks):
                    c_tasks[ci]()
                    ci += 1
            mm_group(s, tsub, 0)
        while ci < len(c_tasks):
            c_tasks[ci]()
            ci += 1

        # D: early slabs' half-1 groups
        for s in range(EARLY):
            for tsub in range(TSUB):
                mm_group(s, tsub, 1)

        # E: steady state - both halves per token block, batched store
        for s in range(EARLY, N_SLABS):
            load_slab(s)
            for tsub in range(TSUB):
                osb = out_pool.tile([P, O_SHARD], bf16, name="osb")
                mm_group(s, tsub, 0, osb)
                mm_group(s, tsub, 1, osb)
                row0 = (s * TSUB + tsub) * P
                nc.sync.dma_start(out[bass.ds(row0, P), :], osb[:])

        for p in reversed(ctx_pools):
            p.release()

    nc.compile()
    return nc


def _get_program():
    if "nc" not in _CACHE:
        _CACHE["nc"] = _build_program()
    return _CACHE["nc"]


def _ensure_ntff_hook():
    """Provide antenv.axon_hooks if the image lacks it (profiling only)."""
    import sys
    import types

    try:
        from antenv.axon_hooks import get_axon_ntff_profile_hook  # noqa: F401
        return
    except ImportError:
        pass
    try:
        import antenv
        from trn_agent_boot.trn_boot import _ntff_profile_via_ctypes

        mod = types.ModuleType("antenv.axon_hooks")
        state = {"hook": _ntff_profile_via_ctypes("/opt/axon/libaxon_pjrt.so")}
        mod.get_axon_ntff_profile_hook = lambda: state["hook"]
        mod.set_axon_ntff_profile_hook = lambda h: state.__setitem__("hook", h)
        sys.modules["antenv.axon_hooks"] = mod
        antenv.axon_hooks = mod
    except Exception:
        pass


def kernel(x: np.ndarray, weight: np.ndarray) -> np.ndarray:
    from concourse.bass_utils import run_bass_kernel_spmd

    assert x.shape == (B, S, D_IN) and weight.shape == (D_OUT, D_IN)
    nc = _get_program()

    xT = np.ascontiguousarray(x.reshape(T, D_IN).T)
    in_maps = []
    for c in range(N_CORES):
        w_shard = weight[c * O_SHARD:(c + 1) * O_SHARD]
        in_maps.append({
            "xT": xT,
            "w": w_shard,
            "wT": np.ascontiguousarray(w_shard.T),
        })

    trace = os.environ.get("BL_TRACE", "0") == "1"
    if trace:
        _ensure_ntff_hook()
    res = run_bass_kernel_spmd(nc, in_maps, list(range(N_CORES)), trace=trace)
    _CACHE["last_results"] = res

    parts = [np.asarray(res.results[c]["out"]) for c in range(N_CORES)]
    full = np.concatenate(parts, axis=1).astype(np.float32)  # [T, D_OUT]
    return np.ascontiguousarray(full.reshape(B, S, D_OUT))


# revision 5
# speedup vs baseline: 1.2966x; 1.0307x over previous
"""BitLinear (ternary-quantized linear) Trainium2 kernel, v3.

Computes: scale = clip(mean(|w|, axis=1), 1e-5);  w_q = clip(round(w/scale), -1, 1)
          out = x @ (w_q * scale).T
for x [4, 2048, 2048] f32, w [8192, 2048] f32, out [4, 2048, 8192] f32.

Strategy (8 NeuronCores in a 2x4 token-half x out-quarter grid):
  - Core (th, oq) computes out features [oq*2048, (oq+1)*2048) for tokens
    [th*4096, (th+1)*4096): x traffic halves vs pure output sharding; total
    per-core HBM traffic is 32(x) + 32(w both layouts) + 16(out bf16) = 80 MB.
  - Host feeds x transposed [d_in, tokens] and the w shard in BOTH layouts
    ([o, k] for the bit-exact per-row scale reduction on the DVE, [k, o] for
    quantization in matmul orientation) - layout choices only; all
    arithmetic happens on device.
  - Per-row scale uses the blocked-512 two-stage reduction (bit-exact match
    with the reference's jnp.mean lowering); w_q = (w > s/2) - (w < -s/2)
    equals clip(round(w/scale), -1, 1) exactly. Ternary w_q is exact in
    fp8e4 and bf16.
  - The contraction (2048 = 16 chunks of 128) is split: the first BL_FP8_KC
    (default 14) chunks run as fp8e4 DoubleRow matmuls (2 chunks/matmul, 2x
    PE rate), the rest as bf16 matmuls. x is cast f32->fp8e4/bf16 on the
    scalar engine per 512-token slab. Measured end-to-end error ~1.9e-2
    scale-relative (gate 2e-2).
  - Matmuls run with w_q stationary and x moving, so PSUM is [o, tokens]
    and the per-o scale is per-PARTITION: the epilogue is a scalar-engine
    activation copy (fast PSUM reads), keeping the vector engine free for
    quantization. Output is stored bf16 in [o, t] layout (contiguous rows,
    one 2 MB store per slab); the host transposes/upcasts.
  - DMA is spread over three HWDGE queues (x loads on sync, weight loads on
    gpsimd, stores on vector) so loads never queue behind stores.
  - Weight prologue is a ladder over o-quarters: quarter 0 quantizes first,
    then the first BL_EARLY slabs' quarter-q matmuls run on the PE while
    quarter q+1's scale/quantization fills the DVE gaps.
"""

import os

import numpy as np

B, S, D_IN, D_OUT = 4, 2048, 2048, 8192
T = B * S  # 8192 tokens
N_CORES = 8
TH, OQN = 2, 4  # grid: token halves x out quarters
T_C = T // TH  # 4096 tokens per core
O_SHARD = D_OUT // OQN  # 2048 out features per core
EPS = 1e-05

P = 128
KC = D_IN // P  # 16 contraction chunks
T_SLAB = 512
N_SLABS = T_C // T_SLAB  # 8
N_OT = O_SHARD // P  # 16 o-tiles
OQ = 512  # o-columns per quant quarter
NQ = O_SHARD // OQ  # 4

NF8 = int(os.environ.get("BL_FP8_KC", "14"))  # k-chunks through fp8 DoubleRow
assert 0 <= NF8 <= KC and NF8 % 2 == 0
NB16 = KC - NF8
NPAIR = NF8 // 2
EARLY = int(os.environ.get("BL_EARLY", "5"))
EARLY = max(1, min(EARLY, N_SLABS))

_CACHE = {}


def _build_program():
    import concourse.bass as bass
    import concourse.tile as tile
    from concourse import bacc, mybir
    from concourse.masks import make_identity

    f32 = mybir.dt.float32
    bf16 = mybir.dt.bfloat16
    fp8 = mybir.dt.float8e4
    DR = mybir.MatmulPerfMode.DoubleRow

    nc = bacc.Bacc(
        "TRN2",
        target_bir_lowering=False,
        debug=False,
        num_devices=N_CORES,
    )

    xT = nc.dram_tensor("xT", [D_IN, T_C], f32, kind="ExternalInput")
    w = nc.dram_tensor("w", [O_SHARD, D_IN], f32, kind="ExternalInput")
    wT = nc.dram_tensor("wT", [D_IN, O_SHARD], f32, kind="ExternalInput")
    out = nc.dram_tensor("out", [O_SHARD, T_C], bf16, kind="ExternalOutput")

    xT3 = xT.ap().rearrange("(c p) t -> p c t", p=P)  # [128, 16, 4096]
    wT3 = wT.ap().rearrange("(c p) o -> p c o", p=P)  # [128, 16, 2048]
    out3 = out.ap().rearrange("(ot p) t -> p ot t", p=P)  # [128, 16, 4096]

    with tile.TileContext(nc) as tc:
        const_pool = tc.alloc_tile_pool(name="const", bufs=1)
        wq8_pool = tc.alloc_tile_pool(name="wq8", bufs=1)
        wq16_pool = tc.alloc_tile_pool(name="wq16", bufs=1)
        sb_pool = tc.alloc_tile_pool(name="thr", bufs=1)
        w_pool = tc.alloc_tile_pool(name="wstage", bufs=2)
        wt_pool = tc.alloc_tile_pool(name="wtstage", bufs=3)
        gl_pool = tc.alloc_tile_pool(name="gl", bufs=2)
        st_pool = tc.alloc_tile_pool(name="stats", bufs=4)
        psum_pro = tc.alloc_tile_pool(name="psum_pro", bufs=1, space="PSUM")
        xb_pool = tc.alloc_tile_pool(name="xb", bufs=EARLY + 1)
        xf_pool = tc.alloc_tile_pool(name="xf", bufs=3)
        out_pool = tc.alloc_tile_pool(name="osb", bufs=1)
        outh_pool = tc.alloc_tile_pool(name="osbh", bufs=3)
        psum_mm = tc.alloc_tile_pool(name="psum_mm", bufs=3, space="PSUM")
        ctx_pools = [const_pool, wq8_pool, wq16_pool, sb_pool, w_pool, wt_pool,
                     gl_pool, st_pool, psum_pro, xb_pool, xf_pool, out_pool,
                     outh_pool, psum_mm]

        ident_f32 = const_pool.tile([P, P], f32)
        make_identity(nc, ident_f32)
        ones_f32 = const_pool.tile([P, P], f32)
        nc.vector.memset(ones_f32[:], 1.0)

        # resident: ternary weights in [k, o] matmul orientation (fp8 chunks
        # + bf16 tail chunks), per-o-tile scale columns, and the +-threshold
        # broadcast across partitions (for quantization compares)
        wqT8 = wq8_pool.tile([P, NF8, O_SHARD], fp8, name="wqT8")
        wqT16 = (wq16_pool.tile([P, NB16, O_SHARD], bf16, name="wqT16")
                 if NB16 else None)
        scales = sb_pool.tile([P, N_OT], f32, tag="scales", name="scales")
        thrB = sb_pool.tile([P, O_SHARD], f32, tag="thrB", name="thrB")
        nthrB = sb_pool.tile([P, O_SHARD], f32, tag="nthrB", name="nthrB")

        def prologue_otile(ot):
            """Bit-exact per-row scale for o-tile `ot` + threshold broadcast."""
            wf = w_pool.tile([P, D_IN], f32, name="wf")
            nc.gpsimd.dma_start(wf[:], w[bass.ts(ot, P), :])

            # blocked-512 two-stage reduce: bit-exact match with the
            # reference's lowered jnp.mean
            ssum4 = st_pool.tile([P, 4], f32, tag="ssum4", name="ssum4")
            nc.vector.tensor_reduce(
                out=ssum4[:],
                in_=wf[:].rearrange("p (b k) -> p b k", k=512),
                op=mybir.AluOpType.add,
                axis=mybir.AxisListType.X,
                apply_absolute_value=True,
            )
            ssum = st_pool.tile([P, 1], f32, tag="ssum", name="ssum")
            nc.vector.tensor_reduce(
                out=ssum[:], in_=ssum4[:],
                op=mybir.AluOpType.add, axis=mybir.AxisListType.X,
            )
            nc.vector.tensor_scalar(
                scales[:, bass.ds(ot, 1)], ssum[:], 1.0 / D_IN, EPS,
                mybir.AluOpType.mult, mybir.AluOpType.max,
            )
            thr = st_pool.tile([P, 1], f32, tag="thr", name="thr")
            nc.vector.tensor_scalar_mul(thr[:], scales[:, bass.ds(ot, 1)], 0.5)

            # thrB[:, ot*128:+128] = thr broadcast over partitions
            # (ones.T @ diag(thr)), likewise -thr into nthrB
            for sign, dst in ((1.0, thrB), (-1.0, nthrB)):
                ds_t = st_pool.tile([P, P], f32, tag=f"diag{sign}", name="ds_t")
                nc.vector.tensor_scalar(
                    ds_t[:], ident_f32[:], thr[:], sign,
                    mybir.AluOpType.mult, mybir.AluOpType.mult,
                )
                bp = psum_pro.tile([P, P], f32, tag="bp", name="bp", bufs=2)
                nc.tensor.matmul(bp[:], ones_f32[:], ds_t[:],
                                 start=True, stop=True)
                nc.scalar.copy(out=dst[:, bass.ts(ot, P)], in_=bp[:])

        def quant_tile(q, kc):
            """Quantize wT chunk kc, o-quarter q, into wqT8/wqT16."""
            qc = bass.ds(q * OQ, OQ)
            wtf = wt_pool.tile([P, OQ], f32, name="wtf")
            nc.gpsimd.dma_start(wtf[:], wT3[:, kc, qc])
            g = gl_pool.tile([P, OQ], f32, tag="g", name="g")
            nc.vector.tensor_tensor(g[:], wtf[:], thrB[:, qc],
                                    mybir.AluOpType.is_gt)
            l = gl_pool.tile([P, OQ], f32, tag="l", name="l")
            nc.vector.tensor_tensor(l[:], wtf[:], nthrB[:, qc],
                                    mybir.AluOpType.is_lt)
            dst = wqT8[:, kc, qc] if kc < NF8 else wqT16[:, kc - NF8, qc]
            nc.vector.tensor_tensor(dst, g[:], l[:], mybir.AluOpType.subtract)

        xb_tiles = {}

        def load_slab(s):
            tsl = bass.ts(s, T_SLAB)
            xb8 = xb_pool.tile([P, NF8, T_SLAB], fp8, tag="xb8", name="xb8")
            xb16 = (xb_pool.tile([P, NB16, T_SLAB], bf16, tag="xb16",
                                 name="xb16") if NB16 else None)
            for quar in range(4):
                xf = xf_pool.tile([P, 4, T_SLAB], f32, name="xf")
                nc.sync.dma_start(xf[:], xT3[:, bass.ts(quar, 4), tsl])
                base = quar * 4
                n8 = max(0, min(4, NF8 - base))
                if n8:
                    nc.scalar.copy(out=xb8[:, bass.ds(base, n8), :],
                                   in_=xf[:, bass.ds(0, n8), :])
                if n8 < 4:
                    nc.scalar.copy(
                        out=xb16[:, bass.ds(base + n8 - NF8, 4 - n8), :],
                        in_=xf[:, bass.ds(n8, 4 - n8), :])
            xb_tiles[s] = (xb8, xb16)

        def mm_group(ot, s, ps, side):
            """All matmuls for o-tile `ot` x token slab `s` (512 tokens)."""
            xb8, xb16 = xb_tiles[s]
            otc = bass.ts(ot, P)
            dst = ps[:, bass.ds(side * T_SLAB, T_SLAB)]
            n_mm = NPAIR + NB16
            idx = 0
            for kp in range(NPAIR):
                nc.tensor.matmul(
                    dst,
                    wqT8[:, bass.ds(2 * kp, 2), otc],
                    xb8[:, bass.ds(2 * kp, 2), :],
                    start=(idx == 0),
                    stop=(idx == n_mm - 1),
                    perf_mode=DR,
                )
                idx += 1
            for j in range(NB16):
                nc.tensor.matmul(
                    dst,
                    wqT16[:, j, otc],
                    xb16[:, j, :],
                    start=(idx == 0),
                    stop=(idx == n_mm - 1),
                )
                idx += 1

        def ot_pair(ot, s, osb, slot):
            """Two o-tiles x one slab through one 2-bank psum tile."""
            ps = psum_mm.tile([P, 2 * T_SLAB], f32, tag="ps", name="ps")
            mm_group(ot, s, ps, 0)
            mm_group(ot + 1, s, ps, 1)
            for i in (0, 1):
                nc.scalar.mul(osb[:, slot + i, :],
                              ps[:, bass.ds(i * T_SLAB, T_SLAB)],
                              scales[:, bass.ds(ot + i, 1)])

        # ---------------- emission schedule -----------------------------
        load_slab(0)
        for ot in range(4):
            prologue_otile(ot)
        for kc in range(KC):
            quant_tile(0, kc)
        for s in range(1, EARLY):
            load_slab(s)

        # Ladder over o-quarters: run the early slabs' quarter-q groups
        # while quarter q+1's prologue fills the DVE gaps.
        phase_tasks = {
            0: ([lambda ot=ot: prologue_otile(ot) for ot in range(4, 8)]
                + [lambda kc=kc: quant_tile(1, kc) for kc in range(KC)]),
            1: ([lambda ot=ot: prologue_otile(ot) for ot in range(8, 16)]
                + [lambda kc=kc: quant_tile(2, kc) for kc in range(KC)]),
            2: [lambda kc=kc: quant_tile(3, kc) for kc in range(KC)],
            3: [],
        }
        for q in range(NQ):
            tasks = phase_tasks[q]
            units = [(s, 4 * q + 2 * pi) for s in range(EARLY)
                     for pi in range(2)]
            per = (len(tasks) + len(units) - 1) // len(units)
            ci = 0
            osbs = {}
            for (s, ot) in units:
                for _ in range(per):
                    if ci < len(tasks):
                        tasks[ci]()
                        ci += 1
                if s not in osbs:
                    osbs[s] = outh_pool.tile([P, 4, T_SLAB], bf16,
                                             name="osbh")
                ot_pair(ot, s, osbs[s], ot - 4 * q)
                if ot % 4 == 2:  # second pair of the quarter for this slab
                    nc.scalar.dma_start(
                        out3[:, bass.ds(4 * q, 4), bass.ts(s, T_SLAB)],
                        osbs.pop(s)[:])
            while ci < len(tasks):
                tasks[ci]()
                ci += 1

        # steady state: all 16 o-tiles per slab, one batched 2 MB store
        for s in range(EARLY, N_SLABS):
            load_slab(s)
            osb = out_pool.tile([P, N_OT, T_SLAB], bf16, name="osb")
            for pi in range(N_OT // 2):
                ot_pair(2 * pi, s, osb, 2 * pi)
            nc.scalar.dma_start(out3[:, :, bass.ts(s, T_SLAB)], osb[:])

        for p in reversed(ctx_pools):
            p.release()

    nc.compile()
    return nc


def _get_program():
    if "nc" not in _CACHE:
        _CACHE["nc"] = _build_program()
    return _CACHE["nc"]


def _ensure_ntff_hook():
    """Provide antenv.axon_hooks if the image lacks it (profiling only)."""
    import sys
    import types

    try:
        from antenv.axon_hooks import get_axon_ntff_profile_hook  # noqa: F401
        return
    except ImportError:
        pass
    try:
        import antenv
        from trn_agent_boot.trn_boot import _ntff_profile_via_ctypes

        mod = types.ModuleType("antenv.axon_hooks")
        state = {"hook": _ntff_profile_via_ctypes("/opt/axon/libaxon_pjrt.so")}
        mod.get_axon_ntff_profile_hook = lambda: state["hook"]
        mod.set_axon_ntff_profile_hook = lambda h: state.__setitem__("hook", h)
        sys.modules["antenv.axon_hooks"] = mod
        antenv.axon_hooks = mod
    except Exception:
        pass


def kernel(x: np.ndarray, weight: np.ndarray) -> np.ndarray:
    from concourse.bass_utils import run_bass_kernel_spmd

    assert x.shape == (B, S, D_IN) and weight.shape == (D_OUT, D_IN)
    nc = _get_program()

    xT = np.ascontiguousarray(x.reshape(T, D_IN).T)  # [D_IN, T]
    in_maps = []
    for c in range(N_CORES):
        th, oq = divmod(c, OQN)
        w_shard = weight[oq * O_SHARD:(oq + 1) * O_SHARD]
        in_maps.append({
            "xT": np.ascontiguousarray(xT[:, th * T_C:(th + 1) * T_C]),
            "w": w_shard,
            "wT": np.ascontiguousarray(w_shard.T),
        })

    trace = os.environ.get("BL_TRACE", "0") == "1"
    if trace:
        _ensure_ntff_hook()
    res = run_bass_kernel_spmd(nc, in_maps, list(range(N_CORES)), trace=trace)
    _CACHE["last_results"] = res

    fullT = np.empty((D_OUT, T), dtype=np.float32)  # [o, t]
    for c in range(N_CORES):
        th, oq = divmod(c, OQN)
        part = np.asarray(res.results[c]["out"]).astype(np.float32)
        fullT[oq * O_SHARD:(oq + 1) * O_SHARD, th * T_C:(th + 1) * T_C] = part
    return np.ascontiguousarray(fullT.T.reshape(B, S, D_OUT))
